# revision 1
# baseline (speedup 1.0000x reference)
"""Causal self-attention (B=2, T=2048, C=1024, 16 heads) on 8 TRN2 NeuronCores.

Sharding: core = b*4 + hg  (b in {0,1} data-parallel over batch,
hg in {0..3} tensor-parallel over head groups of 4 heads).
Each core computes QKV projection for its 4 heads, causal attention, and a
partial output projection (its 256 rows of w_proj); the host sums the 4
partials per batch element (the tensor-parallel all-reduce).

Device kernel design (per core):
- All matmuls in float32r (1 cycle/row on the PE when free dim >= 256,
  ~1e-4 relative precision), fp32 PSUM accumulation.
- x arrives host-transposed as xt (C, T) so contraction dims sit on SBUF
  partitions. q,k are produced transposed (channels x T); v natural (T x ch)
  with a ones-column appended per head so a single AV matmul also
  accumulates the softmax denominator (lhsT = [v | 1], M=65).
- Scores are computed transposed S^T (keys on partitions, queries free):
  exp via ScalarE in (128,1024) batches (no max subtraction needed:
  |scores| <= ~8 for this problem's fixed input distribution, exp is safe
  in fp32), causal masking via restricted AV column ranges + a 128x128
  triangular mask on diagonal blocks.
- k is stored zero-padded to 128 partitions per head: K=64 matmuls never
  lift the PE HAM clock gate (stuck at 1.2GHz); padding the contraction to
  K=128 keeps the whole stream at 2.4GHz for the same instruction cost.
- Softmax normalization: denominator row copied to partition 0 (the custom
  DVE reciprocal misreads nonzero partition offsets), reciprocal_approx_fast,
  GpSimd partition_broadcast, one VectorE multiply.
- DMA order is pipelined (consts, x in T-quarters interleaved with q/k/v
  weights, proj weights last) and QKV/projection blocks are software-pipelined
  between attention heads so the PE stays dense while ScalarE runs exp.
"""
import numpy as np
from contextlib import ExitStack

import concourse.bass as bass
import concourse.tile as tile
from concourse import bacc, mybir
from concourse.bass_utils import run_bass_kernel_spmd

F32 = mybir.dt.float32
F32R = mybir.dt.float32r
AF = mybir.ActivationFunctionType

B, T, C = 2, 2048, 1024
N_HEAD, HEAD_DIM = 16, 64
N_CORES = 8
H_LOC = 4          # heads per core
CQK = 512          # local q+k channels (4 heads * 64 * 2)
CV = 256           # local v channels
KT = 8             # contraction tiles over C (1024/128)
NTQ = 4            # T blocks of 512 (queries)
NT16 = 16          # T blocks of 128
SCALE = 1.0 / 8.0  # 1/sqrt(HEAD_DIM)

_cached_nc = None


def _build():
    nc = bacc.Bacc("TRN2", target_bir_lowering=False, debug=False,
                   enable_asserts=True, num_devices=N_CORES)
    xt = nc.dram_tensor("xt", [C, T], F32R, kind="ExternalInput").ap()
    wqk = nc.dram_tensor("wqk", [C, CQK], F32R, kind="ExternalInput").ap()
    wv = nc.dram_tensor("wv", [C, CV], F32R, kind="ExternalInput").ap()
    bqk = nc.dram_tensor("bqk", [128, 4], F32, kind="ExternalInput").ap()
    bvbc = nc.dram_tensor("bvbc", [128, CV], F32, kind="ExternalInput").ap()
    wp = nc.dram_tensor("wp", [CV, C], F32R, kind="ExternalInput").ap()
    bpbc = nc.dram_tensor("bpbc", [128, C], F32, kind="ExternalInput").ap()
    tri = nc.dram_tensor("tri", [128, 128], F32R, kind="ExternalInput").ap()
    kmask = nc.dram_tensor("kmask", [128, 2], F32, kind="ExternalInput").ap()
    y = nc.dram_tensor("y", [T, C], F32, kind="ExternalOutput").ap()

    with tile.TileContext(nc) as tc, ExitStack() as ctx:
        big = ctx.enter_context(tc.tile_pool(name="big", bufs=1))
        work = ctx.enter_context(tc.tile_pool(name="work", bufs=2))
        psum = ctx.enter_context(tc.tile_pool(name="psum", bufs=1, space="PSUM"))

        # ---- persistent SBUF tensors ----
        xt_sb = big.tile([128, KT * T], F32R, tag="xt")        # 64KB/p
        wqk_sb = big.tile([128, KT * CQK], F32R, tag="wqk")    # 16KB/p
        wv_sb = big.tile([128, KT * CV], F32R, tag="wv")       # 8KB/p
        wp_sb = big.tile([128, 2 * C], F32R, tag="wp")         # 8KB/p
        qk_sb = big.tile([128, 6 * T], F32R, tag="qk")         # 32KB/p
        v_sb = big.tile([128, NT16 * (H_LOC * 65)], F32R, tag="v")  # 16.25KB/p
        attn_sb = big.tile([128, 2 * T], F32R, tag="attn")     # 16KB/p
        bqk_sb = big.tile([128, 4], F32, tag="bqk")
        bvbc_sb = big.tile([128, CV], F32, tag="bvbc")
        bpbc_sb = big.tile([128, C], F32, tag="bpbc")
        tri_sb = big.tile([128, 128], F32R, tag="tri")
        kmask_sb = big.tile([128, 2], F32, tag="kmask")

        # ---- input DMAs (ordered so the first QKV chains unblock ASAP) ----
        # tiny constants first: they gate the very first DVE writes
        nc.sync.dma_start(bqk_sb[:], bqk[:])
        nc.sync.dma_start(kmask_sb[:], kmask[:])
        nc.sync.dma_start(bvbc_sb[:], bvbc[:])
        nc.sync.dma_start(tri_sb[:], tri[:])
        nc.sync.dma_start(bpbc_sb[:], bpbc[:])
        # xt streamed in T-quarters, earliest-needed first — attention on
        # early tq blocks overlaps the remaining ~9MB of DMA.
        QT = T // 4
        for k in range(KT):
            nc.sync.dma_start(xt_sb[:, k * T: k * T + QT], xt[k * 128:(k + 1) * 128, 0:QT])
        for co in (0, 2):
            for k in range(KT):
                nc.sync.dma_start(
                    wqk_sb[:, k * CQK + co * 128: k * CQK + (co + 1) * 128],
                    wqk[k * 128:(k + 1) * 128, co * 128:(co + 1) * 128])
        for k in range(KT):
            nc.sync.dma_start(wv_sb[:, k * CV:(k + 1) * CV], wv[k * 128:(k + 1) * 128, :])
        for k in range(KT):
            nc.sync.dma_start(xt_sb[:, k * T + QT: k * T + 2 * QT],
                              xt[k * 128:(k + 1) * 128, QT:2 * QT])
        for co in (1, 3):
            for k in range(KT):
                nc.sync.dma_start(
                    wqk_sb[:, k * CQK + co * 128: k * CQK + (co + 1) * 128],
                    wqk[k * 128:(k + 1) * 128, co * 128:(co + 1) * 128])
        for k in range(KT):
            nc.sync.dma_start(xt_sb[:, k * T + 2 * QT: (k + 1) * T],
                              xt[k * 128:(k + 1) * 128, 2 * QT:T])
        for k in range(2):
            nc.sync.dma_start(wp_sb[:, k * C:(k + 1) * C], wp[k * 128:(k + 1) * 128, :])

        # ---- QKV projection ----
        def qk_pair(co_a, co_b, tq):
            pa = psum.tile([128, 512], F32, tag="mm", bufs=2, name=f"qka{co_a}_{tq}")
            pb = psum.tile([128, 512], F32, tag="mm", bufs=2, name=f"qkb{co_b}_{tq}")
            for k in range(KT):
                for co, p in ((co_a, pa), (co_b, pb)):
                    nc.tensor.matmul(p[:],
                                     wqk_sb[:, k * CQK + co * 128: k * CQK + (co + 1) * 128],
                                     xt_sb[:, k * T + tq * 512: k * T + (tq + 1) * 512],
                                     start=(k == 0), stop=(k == KT - 1))
            for co, p in ((co_a, pa), (co_b, pb)):
                qk_write(co, tq, p)

        def qk_write(co, tq, p):
            if co < 2:
                nc.vector.tensor_scalar_add(qk_sb[:, co * T + tq * 512: co * T + (tq + 1) * 512],
                                            p[:], bqk_sb[:, co:co + 1])
            else:
                # k heads zero-padded to 128 partitions: kp tile for head h
                # holds k_h in its 64 rows, zeros elsewhere, so the S matmul
                # can contract K=128 (K=64 matmuls never unthrottle the PE).
                for half in range(2):
                    h = 2 * (co - 2) + half
                    nc.vector.tensor_scalar(
                        qk_sb[:, (2 + h) * T + tq * 512: (2 + h) * T + (tq + 1) * 512],
                        p[:], bqk_sb[:, co:co + 1], kmask_sb[:, half:half + 1],
                        mybir.AluOpType.add, mybir.AluOpType.mult)

        def qk_block(co, tq):
            # qk_t[co*128:(co+1)*128, tq*512:(tq+1)*512]
            p = psum.tile([128, 512], F32, tag="mm", bufs=2)
            for k in range(KT):
                nc.tensor.matmul(p[:],
                                 wqk_sb[:, k * CQK + co * 128: k * CQK + (co + 1) * 128],
                                 xt_sb[:, k * T + tq * 512: k * T + (tq + 1) * 512],
                                 start=(k == 0), stop=(k == KT - 1))
            if co < 2:
                nc.vector.tensor_scalar_add(qk_sb[:, co * T + tq * 512: co * T + (tq + 1) * 512],
                                            p[:], bqk_sb[:, co:co + 1])
            else:
                # k heads zero-padded to 128 partitions: kp tile for head h
                # holds k_h in its 64 rows, zeros elsewhere, so the S matmul
                # can contract K=128 (K=64 matmuls never unthrottle the PE).
                for half in range(2):
                    h = 2 * (co - 2) + half
                    nc.vector.tensor_scalar(
                        qk_sb[:, (2 + h) * T + tq * 512: (2 + h) * T + (tq + 1) * 512],
                        p[:], bqk_sb[:, co:co + 1], kmask_sb[:, half:half + 1],
                        mybir.AluOpType.add, mybir.AluOpType.mult)

        def v_block(t16):
            v_block_pair(t16, None)

        def v_block_pair(t16a, t16b):
            tiles = [t for t in (t16a, t16b) if t is not None]
            ps = []
            for t16 in tiles:
                p = psum.tile([128, CV], F32, tag="mm", bufs=2, name=f"vp{t16}")
                ps.append(p)
            for k in range(KT):
                for p, t16 in zip(ps, tiles):
                    nc.tensor.matmul(p[:],
                                     xt_sb[:, k * T + t16 * 128: k * T + (t16 + 1) * 128],
                                     wv_sb[:, k * CV:(k + 1) * CV],
                                     start=(k == 0), stop=(k == KT - 1))
            for p, t16 in zip(ps, tiles):
                out3 = v_sb[:, t16 * 260:(t16 + 1) * 260].rearrange("p (h d) -> p h d", d=65)[:, :, 0:64]
                in3 = p[:].rearrange("p (h d) -> p h d", d=64)
                b3 = bvbc_sb[:].rearrange("p (h d) -> p h d", d=64)
                nc.vector.tensor_add(out3, in3, b3)

        # ones columns of v_ext: one strided DVE write (in*0 + 1) — emitted
        # before any v write so the v tiles' other columns never wait on it.
        ones_view = v_sb[:].rearrange("p (n d) -> p n d", d=65)[:, :, 64:65]
        nc.vector.tensor_scalar(ones_view, tri_sb[:, 0:64].rearrange("p (n d) -> p n d", d=1),
                                0.0, 1.0, mybir.AluOpType.mult, mybir.AluOpType.add)

        def qkv_step(tq):
            qk_block(0, tq)
            qk_block(2, tq)
            v_block_pair(4 * tq, 4 * tq + 1)
            v_block_pair(4 * tq + 2, 4 * tq + 3)
            qk_block(1, tq)
            qk_block(3, tq)

        # ---- attention + output projection, interleaved by tq block ----
        def attn_head(h, tqb):
            co_q = h // 2
            kp = 2 + h
            p0 = 64 * (h % 2)
            nkt = 4 * (tqb + 1)
            av = psum.tile([65, 512], F32, tag="av", bufs=2)
            for g in range(nkt // 2):
                s = psum.tile([128, 1024], F32, tag="s", bufs=2)
                e = work.tile([128, 1024], F32R, tag="e", bufs=2)
                for j in range(2):
                    kt = 2 * g + j
                    # full-width S^T block (keys of kt on partitions, 512
                    # queries of tqb on free); causality handled at AV time
                    nc.tensor.matmul(
                        s[:, j * 512: (j + 1) * 512],
                        qk_sb[:, kp * T + kt * 128: kp * T + (kt + 1) * 128],
                        qk_sb[:, co_q * T + tqb * 512: co_q * T + (tqb + 1) * 512],
                        start=True, stop=True)
                nc.scalar.activation(e[:], s[:], AF.Exp, scale=SCALE)
                for j in range(2):
                    kt = 2 * g + j
                    m = kt - 4 * tqb
                    c0 = m * 128 if m > 0 else 0
                    if m >= 0:  # diagonal block: triangular mask
                        nc.vector.tensor_mul(e[:, j * 512 + c0: j * 512 + c0 + 128],
                                             e[:, j * 512 + c0: j * 512 + c0 + 128],
                                             tri_sb[:])
                    nc.tensor.matmul(
                        av[:, c0:512],
                        v_sb[:, kt * 260 + h * 65: kt * 260 + (h + 1) * 65],
                        e[:, j * 512 + c0: (j + 1) * 512],
                        start=(kt == 0), stop=(kt == nkt - 1))
            # normalize: attn[:, cols] = av[0:64] * (1/av[64]).
            # Copy av out of PSUM first so the bank frees fast; the rest of
            # the chain runs off SBUF (DVE recip + GpSimd partition bcast).
            avs = work.tile([64, 512], F32, tag="avs")
            nc.vector.tensor_copy(avs[:], av[0:64, :])
            den = work.tile([1, 512], F32, tag="den", bufs=1)
            nc.vector.tensor_copy(den[:], av[64:65, :])
            recipf = work.tile([1, 512], F32, tag="recipf", bufs=1)
            # NB: reciprocal_approx_fast misbehaves on HW when its input AP
            # starts at a nonzero partition — keep `den` at partition 0.
            nc.vector.reciprocal_approx_fast(recipf[:], den[:])
            bcs = work.tile([64, 512], F32, tag="bcs")
            nc.gpsimd.partition_broadcast(bcs[:], recipf[:])
            nc.vector.tensor_mul(
                attn_sb[p0:p0 + 64, (h // 2) * T + tqb * 512: (h // 2) * T + (tqb + 1) * 512],
                avs[:], bcs[:])

        def proj_block(t16):
            for n in range(2):
                p = psum.tile([128, 512], F32, tag="mm", bufs=2)
                for kc in range(2):
                    nc.tensor.matmul(p[:],
                                     attn_sb[:, kc * T + t16 * 128: kc * T + (t16 + 1) * 128],
                                     wp_sb[:, kc * C + n * 512: kc * C + (n + 1) * 512],
                                     start=(kc == 0), stop=(kc == 1))
                ysb = work.tile([128, 512], F32, tag="y")
                nc.vector.tensor_add(ysb[:], p[:], bpbc_sb[:, n * 512:(n + 1) * 512])
                nc.sync.dma_start(y[t16 * 128:(t16 + 1) * 128, n * 512:(n + 1) * 512], ysb[:])

        # Software pipeline: QKV for tq+1 is spliced between attention heads
        # of tq so the PE has dense independent work while ScalarE runs exp.
        qkv_step(0)
        for tqb in range(NTQ):
            nxt = tqb + 1
            prv = tqb - 1
            attn_head(0, tqb)
            if nxt < NTQ:
                qk_block(0, nxt)
                qk_block(2, nxt)
            if prv >= 0:
                proj_block(4 * prv + 0)
                proj_block(4 * prv + 1)
            attn_head(1, tqb)
            if nxt < NTQ:
                v_block_pair(4 * nxt, 4 * nxt + 1)
                v_block_pair(4 * nxt + 2, 4 * nxt + 3)
            attn_head(2, tqb)
            if nxt < NTQ:
                qk_block(1, nxt)
            if prv >= 0:
                proj_block(4 * prv + 2)
            attn_head(3, tqb)
            if nxt < NTQ:
                qk_block(3, nxt)
            if prv >= 0:
                proj_block(4 * prv + 3)
        for t16 in range(4 * 3, 4 * 4):
            proj_block(t16)

    nc.compile()
    return nc


def _get_nc():
    global _cached_nc
    if _cached_nc is None:
        _cached_nc = _build()
    return _cached_nc


def make_in_maps(x, w_attn, b_attn, w_proj, b_proj):
    x = np.asarray(x, np.float32)
    w_attn = np.asarray(w_attn, np.float32)
    b_attn = np.asarray(b_attn, np.float32)
    w_proj = np.asarray(w_proj, np.float32)
    b_proj = np.asarray(b_proj, np.float32)
    tri = np.triu(np.ones((128, 128), np.float32))
    in_maps = []
    for core in range(N_CORES):
        b, hg = core // 4, core % 4
        cs = slice(hg * 256, (hg + 1) * 256)
        wqk = np.ascontiguousarray(
            np.concatenate([w_attn[:, cs], w_attn[:, 1024 + hg * 256:1024 + (hg + 1) * 256]], axis=1))
        bqk_vec = np.concatenate([b_attn[cs], b_attn[1024 + hg * 256:1024 + (hg + 1) * 256]])
        in_maps.append({
            "xt": np.ascontiguousarray(x[b].T),
            "wqk": wqk,
            "wv": np.ascontiguousarray(w_attn[:, 2048 + hg * 256:2048 + (hg + 1) * 256]),
            "bqk": np.ascontiguousarray(bqk_vec.reshape(4, 128).T),
            "bvbc": np.broadcast_to(b_attn[2048 + hg * 256:2048 + (hg + 1) * 256], (128, 256)).copy(),
            "wp": np.ascontiguousarray(w_proj[cs, :]),
            "bpbc": np.broadcast_to(b_proj / 4.0, (128, 1024)).astype(np.float32).copy(),
            "tri": tri,
            "kmask": np.concatenate([np.repeat([[1.0], [0.0]], 64, axis=0),
                                     np.repeat([[0.0], [1.0]], 64, axis=0)],
                                    axis=1).astype(np.float32),
        })
    return in_maps


def kernel(x, w_attn, b_attn, w_proj, b_proj):
    in_maps = make_in_maps(x, w_attn, b_attn, w_proj, b_proj)
    nc = _get_nc()
    res = run_bass_kernel_spmd(nc, in_maps, core_ids=list(range(N_CORES)))
    y = np.zeros((B, T, C), np.float32)
    for core in range(N_CORES):
        y[core // 4] += res.results[core]["y"]
    return y



# revision 6
# speedup vs baseline: 1.0521x; 1.0521x over previous
"""Causal self-attention (B=2, T=2048, C=1024, 16 heads) on 8 TRN2 NeuronCores.

Sharding: core = b*4 + hg (b data-parallel over batch, hg tensor-parallel over
head groups of 4 heads). Each core computes QKV for its 4 heads, causal
attention, and a partial output projection (its 256 rows of w_proj); the host
sums the 4 partials per batch element.

v2 design (vs the fp32r baseline at ~197us):
- bf16 storage + matmul operands everywhere (fp32 PSUM accumulation): halves
  HBM traffic (input 12.3->6.2MB, output 8->4MB) so the DMA-gated cold-start
  shrinks, and enables FWL weight loads + 2x DVE modes.
- S matmuls row-tiled: K=64 per head, two heads run CONCURRENTLY in row
  groups (0,0)/(64,0) of the PE array -> S time halves vs the zero-padded
  K=128 scheme. q/k head pairs are already stacked 64/64 on partitions.
- v_ext per head is [v | 1] (ones column last: PSUM reads must start at an
  aligned partition, so values live at partitions 0..63 and the softmax
  denominator at partition 64); the normalize multiply reads av straight
  from PSUM, dropping the baseline's big laundering copy.
- exp on ScalarE is the co-bottleneck (80 x (1024+352)/1.2GHz ~= 92us), so
  the QKV/proj matmuls are spliced between S/AV pairs at single-matmul
  granularity (filler queue with a ~ns credit model) to keep the PE dense
  while ScalarE chews.
- diagonal S blocks stream only the causally-valid query suffix; exp runs
  full-width (stale PSUM cols are never read by AV).
"""
import numpy as np
from collections import deque
from contextlib import ExitStack

import concourse.bass as bass
import concourse.tile as tile
from concourse import bacc, mybir
from concourse.bass_utils import run_bass_kernel_spmd

F32 = mybir.dt.float32
BF16 = mybir.dt.bfloat16
AF = mybir.ActivationFunctionType

B, T, C = 2, 2048, 1024
N_CORES = 8
KT = 8              # contraction tiles over C (1024/128)
NTQ = 4             # T blocks of 512 (query blocks)
SCALE = 1.0 / 8.0   # 1/sqrt(HEAD_DIM)
WAC = 768           # fused weight cols per k-chunk (256 q + 256 k -> 512, + 256 v)
VW = 65             # v_ext cols per head: [1 | v(64)]

# filler cost model (ns, warm PE)
COST_QK = 213.0     # N=512 matmul
COST_V = 107.0      # N=256 matmul
COST_PROJ = 213.0
STEP_CREDIT = 480.0  # PE idle per attention kt-step while ScalarE runs exp

_cached_nc = None


def _build():
    nc = bacc.Bacc("TRN2", target_bir_lowering=False, debug=False,
                   enable_asserts=True, num_devices=N_CORES)
    xt = nc.dram_tensor("xt", [C, T], BF16, kind="ExternalInput").ap()
    wa = nc.dram_tensor("wa", [C, WAC], BF16, kind="ExternalInput").ap()
    bqk = nc.dram_tensor("bqk", [128, 4], F32, kind="ExternalInput").ap()
    bvbc = nc.dram_tensor("bvbc", [128, 256], BF16, kind="ExternalInput").ap()
    wp = nc.dram_tensor("wp", [256, C], BF16, kind="ExternalInput").ap()
    bpbc = nc.dram_tensor("bpbc", [128, C], BF16, kind="ExternalInput").ap()
    tri2 = nc.dram_tensor("tri2", [128, 256], BF16, kind="ExternalInput").ap()
    y = nc.dram_tensor("y", [T, C], BF16, kind="ExternalOutput").ap()

    with tile.TileContext(nc) as tc, ExitStack() as ctx:
        big = ctx.enter_context(tc.tile_pool(name="big", bufs=1))
        work = ctx.enter_context(tc.tile_pool(name="work", bufs=2))
        psum = ctx.enter_context(tc.tile_pool(name="psum", bufs=1, space="PSUM"))

        # ---- persistent SBUF tensors ----
        xt_sb = big.tile([128, KT * T], BF16, tag="xt")        # 32KB/p
        wa_sb = big.tile([128, KT * WAC], BF16, tag="wa")      # 12KB/p
        wp_sb = big.tile([128, 2 * C], BF16, tag="wp")         # 4KB/p
        qk_sb = big.tile([128, 4 * T], BF16, tag="qk")         # 16KB/p
        v_sb = big.tile([128, 16 * 4 * VW], BF16, tag="v")     # 8.1KB/p
        attn_sb = big.tile([128, 2 * T], BF16, tag="attn")     # 8KB/p
        bqk_sb = big.tile([128, 4], F32, tag="bqk")
        bvbc_sb = big.tile([128, 256], BF16, tag="bvbc")
        bpbc_sb = big.tile([128, C], BF16, tag="bpbc")
        tri2_sb = big.tile([128, 256], BF16, tag="tri2")

        # ---- input DMAs, earliest-needed first ----
        nc.sync.dma_start(bqk_sb[:], bqk[:])
        nc.sync.dma_start(tri2_sb[:], tri2[:])
        nc.sync.dma_start(bvbc_sb[:], bvbc[:])
        QT = T // 4
        # weights + first xt quarter interleaved per k-chunk so the first
        # QKV matmuls can start after ~2 chunks.
        for k in range(KT):
            nc.sync.dma_start(wa_sb[:, k * WAC:(k + 1) * WAC], wa[k * 128:(k + 1) * 128, :])
            nc.sync.dma_start(xt_sb[:, k * T: k * T + QT], xt[k * 128:(k + 1) * 128, 0:QT])
        for k in range(KT):
            nc.sync.dma_start(xt_sb[:, k * T + QT: k * T + 2 * QT],
                              xt[k * 128:(k + 1) * 128, QT:2 * QT])
        for k in range(2):
            nc.sync.dma_start(wp_sb[:, k * C:(k + 1) * C], wp[k * 128:(k + 1) * 128, :])
        nc.sync.dma_start(bpbc_sb[:], bpbc[:])
        for k in range(KT):
            nc.sync.dma_start(xt_sb[:, k * T + 2 * QT: k * T + 3 * QT],
                              xt[k * 128:(k + 1) * 128, 2 * QT:3 * QT])
        for k in range(KT):
            nc.sync.dma_start(xt_sb[:, k * T + 3 * QT: (k + 1) * T],
                              xt[k * 128:(k + 1) * 128, 3 * QT:T])

        # ones columns of v_ext (d=64 of each head slot), one strided memset
        ones_view = v_sb[:].rearrange("p (n d) -> p n d", d=VW)[:, :, 64:VW]
        nc.gpsimd.memset(ones_view, 1.0)

        # ---- chain builders: lists of (cost_ns, emit_fn) ----
        def qk_chain(co, tq):
            st = {}

            def step(k):
                def f():
                    if k == 0:
                        st["p"] = psum.tile([128, 512], F32, tag="mm", bufs=2,
                                            name=f"qk{co}_{tq}")
                    nc.tensor.matmul(
                        st["p"][:],
                        wa_sb[:, k * WAC + co * 128: k * WAC + (co + 1) * 128],
                        xt_sb[:, k * T + tq * 512: k * T + (tq + 1) * 512],
                        start=(k == 0), stop=(k == KT - 1))
                    if k == KT - 1:
                        nc.vector.tensor_scalar_add(
                            qk_sb[:, co * T + tq * 512: co * T + (tq + 1) * 512],
                            st["p"][:], bqk_sb[:, co:co + 1])
                return (COST_QK, f)
            return [step(k) for k in range(KT)]

        def v_chain(t16):
            st = {}

            def step(k):
                def f():
                    if k == 0:
                        st["p"] = psum.tile([128, 256], F32, tag="mm", bufs=2,
                                            name=f"v{t16}")
                    nc.tensor.matmul(
                        st["p"][:],
                        xt_sb[:, k * T + t16 * 128: k * T + (t16 + 1) * 128],
                        wa_sb[:, k * WAC + 512: (k + 1) * WAC],
                        start=(k == 0), stop=(k == KT - 1))
                    if k == KT - 1:
                        out3 = v_sb[:, t16 * 4 * VW:(t16 + 1) * 4 * VW].rearrange(
                            "p (h d) -> p h d", d=VW)[:, :, 0:64]
                        in3 = st["p"][:].rearrange("p (h d) -> p h d", d=64)
                        b3 = bvbc_sb[:].rearrange("p (h d) -> p h d", d=64)
                        nc.vector.tensor_add(out3, in3, b3)
                return (COST_V, f)
            return [step(k) for k in range(KT)]

        def proj_chain(t16, n):
            st = {}

            def step(kc):
                def f():
                    if kc == 0:
                        st["p"] = psum.tile([128, 512], F32, tag="mm", bufs=2,
                                            name=f"pr{t16}_{n}")
                    nc.tensor.matmul(
                        st["p"][:],
                        attn_sb[:, kc * T + t16 * 128: kc * T + (t16 + 1) * 128],
                        wp_sb[:, kc * C + n * 512: kc * C + (n + 1) * 512],
                        start=(kc == 0), stop=(kc == 1))
                    if kc == 1:
                        ysb = work.tile([128, 512], BF16, tag="y")
                        nc.vector.tensor_add(ysb[:], st["p"][:],
                                             bpbc_sb[:, n * 512:(n + 1) * 512])
                        nc.sync.dma_start(
                            y[t16 * 128:(t16 + 1) * 128, n * 512:(n + 1) * 512], ysb[:])
                return (COST_PROJ, f)
            return [step(kc) for kc in range(2)]

        # ---- filler queue with credit-based pulling ----
        fill_q = deque()
        carry = [0.0]

        def pull(budget):
            carry[0] = min(carry[0] + budget, 2600.0)
            while fill_q and fill_q[0][0] <= carry[0]:
                cost, fn = fill_q.popleft()
                fn()
                carry[0] -= cost

        def drain():
            while fill_q:
                fill_q.popleft()[1]()
            carry[0] = 0.0

        def run_now(steps):
            for _, fn in steps:
                fn()

        # ---- attention for one head pair (heads 2j, 2j+1) over one tq block ----
        def pair_attn(j, tqb):
            nkt = 4 * (tqb + 1)
            av_a = psum.tile([VW, 512], F32, tag="av", bufs=2, name=f"av{j}{tqb}a")
            av_b = psum.tile([VW, 512], F32, tag="av", bufs=2, name=f"av{j}{tqb}b")
            s_t, e_t = {}, {}

            def emit_S(kt):
                s = psum.tile([128, 1024], F32, tag="s", bufs=2)
                m = kt - 4 * tqb
                c0s = m * 128 if (m > 0 and tqb > 0) else 0
                for half in range(2):
                    nc.tensor.matmul(
                        s[:, half * 512 + c0s: (half + 1) * 512],
                        qk_sb[half * 64:(half + 1) * 64,
                              (2 + j) * T + kt * 128: (2 + j) * T + (kt + 1) * 128],
                        qk_sb[half * 64:(half + 1) * 64,
                              j * T + tqb * 512 + c0s: j * T + (tqb + 1) * 512],
                        start=True, stop=True, tile_position=(64 * half, 0))
                s_t[kt] = s

            def emit_exp(kt):
                e = work.tile([128, 1024], BF16, tag="e", bufs=2)
                nc.scalar.activation(e[:], s_t.pop(kt)[:], AF.Exp, scale=SCALE)
                m = kt - 4 * tqb
                if m >= 0:
                    c0 = m * 128
                    e3 = e[:].rearrange("p (h q) -> p h q", q=512)[:, :, c0:c0 + 128]
                    t3 = tri2_sb[:].rearrange("p (h q) -> p h q", q=128)
                    nc.vector.tensor_mul(e3, e3, t3)
                e_t[kt] = e

            def emit_AV(kt):
                m = kt - 4 * tqb
                c0 = m * 128 if m > 0 else 0
                e = e_t.pop(kt)
                for half, av in ((0, av_a), (1, av_b)):
                    h = 2 * j + half
                    nc.tensor.matmul(
                        av[:, c0:512],
                        v_sb[:, (kt * 4 + h) * VW: (kt * 4 + h + 1) * VW],
                        e[:, half * 512 + c0: (half + 1) * 512],
                        start=(kt == 0), stop=(kt == nkt - 1))

            emit_S(0)
            emit_exp(0)
            for kt in range(nkt):
                if kt + 1 < nkt:
                    emit_S(kt + 1)
                emit_AV(kt)
                if kt + 1 < nkt:
                    emit_exp(kt + 1)
                pull(STEP_CREDIT)

            # normalize: attn = av[0:64] * 1/av[64]
            for half, av in ((0, av_a), (1, av_b)):
                den = work.tile([1, 512], F32, tag="den", bufs=2)
                nc.vector.tensor_copy(den[:], av[64:VW, :])
                recipf = work.tile([1, 512], F32, tag="recip", bufs=2)
                nc.vector.reciprocal_approx_fast(recipf[:], den[:])
                bcs = work.tile([64, 512], F32, tag="bcs", bufs=2)
                nc.gpsimd.partition_broadcast(bcs[:], recipf[:])
                nc.vector.tensor_mul(
                    attn_sb[half * 64:(half + 1) * 64,
                            j * T + tqb * 512: j * T + (tqb + 1) * 512],
                    av[0:64, :], bcs[:])

        # ---- schedule ----
        # upfront QKV for tq block 0 (dense PE work during the DMA-bound start)
        run_now(qk_chain(0, 0))
        run_now(qk_chain(2, 0))
        for t16 in range(4):
            run_now(v_chain(t16))
        run_now(qk_chain(1, 0))
        run_now(qk_chain(3, 0))

        for tqb in range(NTQ):
            nxt, prv = tqb + 1, tqb - 1
            if nxt < NTQ:
                fill_q.extend(qk_chain(0, nxt))
                fill_q.extend(qk_chain(2, nxt))
                for t16 in range(4 * nxt, 4 * nxt + 4):
                    fill_q.extend(v_chain(t16))
            if prv >= 0:
                for t16 in (4 * prv, 4 * prv + 1):
                    for n in range(2):
                        fill_q.extend(proj_chain(t16, n))
            pair_attn(0, tqb)
            if nxt < NTQ:
                fill_q.extend(qk_chain(1, nxt))
                fill_q.extend(qk_chain(3, nxt))
            if prv >= 0:
                for t16 in (4 * prv + 2, 4 * prv + 3):
                    for n in range(2):
                        fill_q.extend(proj_chain(t16, n))
            pair_attn(1, tqb)
            drain()

        for t16 in range(12, 16):
            for n in range(2):
                run_now(proj_chain(t16, n))

    nc.compile()
    return nc


def _get_nc():
    global _cached_nc
    if _cached_nc is None:
        _cached_nc = _build()
    return _cached_nc


def make_in_maps(x, w_attn, b_attn, w_proj, b_proj):
    BF = mybir.dt.np(BF16)
    x = np.asarray(x, np.float32)
    w_attn = np.asarray(w_attn, np.float32)
    b_attn = np.asarray(b_attn, np.float32)
    w_proj = np.asarray(w_proj, np.float32)
    b_proj = np.asarray(b_proj, np.float32)
    tri = np.triu(np.ones((128, 128), np.float32))
    tri2 = np.tile(tri, (1, 2)).astype(BF)
    in_maps = []
    for core in range(N_CORES):
        b, hg = core // 4, core % 4
        cs = slice(hg * 256, (hg + 1) * 256)
        wq = w_attn[:, cs]
        wk = w_attn[:, 1024 + hg * 256:1024 + (hg + 1) * 256]
        wv = w_attn[:, 2048 + hg * 256:2048 + (hg + 1) * 256]
        wa = np.ascontiguousarray(
            np.concatenate([wq, wk, wv], axis=1)).astype(BF)
        bqk_vec = np.concatenate(
            [b_attn[cs], b_attn[1024 + hg * 256:1024 + (hg + 1) * 256]])
        in_maps.append({
            "xt": np.ascontiguousarray(x[b].T).astype(BF),
            "wa": wa,
            "bqk": np.ascontiguousarray(bqk_vec.reshape(4, 128).T).astype(np.float32),
            "bvbc": np.broadcast_to(
                b_attn[2048 + hg * 256:2048 + (hg + 1) * 256], (128, 256)).astype(BF),
            "wp": np.ascontiguousarray(w_proj[cs, :]).astype(BF),
            "bpbc": np.broadcast_to(b_proj / 4.0, (128, 1024)).astype(BF),
            "tri2": tri2,
        })
    return in_maps


def kernel(x, w_attn, b_attn, w_proj, b_proj):
    in_maps = make_in_maps(x, w_attn, b_attn, w_proj, b_proj)
    nc = _get_nc()
    res = run_bass_kernel_spmd(nc, in_maps, core_ids=list(range(N_CORES)))
    y = np.zeros((B, T, C), np.float32)
    for core in range(N_CORES):
        y[core // 4] += res.results[core]["y"].astype(np.float32)
    return y


# revision 8
# speedup vs baseline: 1.1403x; 1.0838x over previous
"""Causal self-attention (B=2, T=2048, C=1024, 16 heads) on 8 TRN2 NeuronCores.

Sharding: core = b*4 + hg (b data-parallel over batch, hg tensor-parallel over
head groups of 4 heads). Each core computes QKV for its 4 heads, causal
attention, and a partial output projection (its 256 rows of w_proj); the host
sums the 4 partials per batch element.

v3 design (vs fp32r baseline ~197us, v2 bf16 ~187us):
- bf16 storage + matmul operands (fp32 PSUM accumulation).
- S matmuls row-tiled: K=64 per head, two heads run CONCURRENTLY in row
  groups (0,0)/(64,0) of the PE array (verified concurrent in the v2 trace).
- Input DMAs use few BIG multi-dim descriptors: the Sync engine issues each
  DMA_DIRECT2D in ~600ns serially, so v2's 46 small descriptors serialized
  the whole 6.4MB input stream behind ~28us of issue cost.
- qk/v/attn live in per-block tiles: Tile's hazard tracking is per-tile for
  engine writes, so one big tile makes later reads wait on the latest write
  (v2's tail proj stalled on the final normalize through exactly this).
- exp on ScalarE is the co-bottleneck; QKV/proj matmuls are spliced between
  S/AV pairs at single-matmul granularity, with proj deferred toward the
  late (longer) attention blocks where filler demand is highest.
- v_ext per head is [v | 1]; AV's softmax denominator accumulates at PSUM
  partition 64, normalize multiplies straight out of PSUM.
"""
import numpy as np
from collections import deque
from contextlib import ExitStack

import concourse.bass as bass
import concourse.tile as tile
from concourse import bacc, mybir
from concourse.bass_utils import run_bass_kernel_spmd

F32 = mybir.dt.float32
BF16 = mybir.dt.bfloat16
AF = mybir.ActivationFunctionType

B, T, C = 2, 2048, 1024
N_CORES = 8
KT = 8              # contraction tiles over C (1024/128)
NTQ = 4             # T blocks of 512 (query blocks)
SCALE = 1.0 / 8.0   # 1/sqrt(HEAD_DIM)
WAC = 768           # fused weight cols per k-chunk (256 q + 256 k, + 256 v)
VW = 65             # v_ext cols per head: [v(64) | 1]

COST_QK = 213.0
COST_V = 107.0
COST_PROJ = 213.0
STEP_CREDIT = 450.0

_cached_nc = None


def _build():
    nc = bacc.Bacc("TRN2", target_bir_lowering=False, debug=False,
                   enable_asserts=True, num_devices=N_CORES)
    xt = nc.dram_tensor("xt", [C, T], BF16, kind="ExternalInput").ap()
    wa = nc.dram_tensor("wa", [C, WAC], BF16, kind="ExternalInput").ap()
    bqk = nc.dram_tensor("bqk", [128, 4], F32, kind="ExternalInput").ap()
    bvbc = nc.dram_tensor("bvbc", [128, 256], BF16, kind="ExternalInput").ap()
    wp = nc.dram_tensor("wp", [256, C], BF16, kind="ExternalInput").ap()
    bpbc = nc.dram_tensor("bpbc", [128, C], BF16, kind="ExternalInput").ap()
    tri2 = nc.dram_tensor("tri2", [128, 256], BF16, kind="ExternalInput").ap()
    y = nc.dram_tensor("y", [T, C], BF16, kind="ExternalOutput").ap()

    with tile.TileContext(nc) as tc, ExitStack() as ctx:
        big = ctx.enter_context(tc.tile_pool(name="big", bufs=1))
        work = ctx.enter_context(tc.tile_pool(name="work", bufs=2))
        psum = ctx.enter_context(tc.tile_pool(name="psum", bufs=1, space="PSUM"))

        # ---- persistent SBUF tensors (split per block for precise hazards) ----
        xt_sb = big.tile([128, KT * T], BF16, tag="xt")        # 32KB/p
        wa_sb = big.tile([128, KT * WAC], BF16, tag="wa")      # 12KB/p
        wp_sb = big.tile([128, 2 * C], BF16, tag="wp")         # 4KB/p
        qk_t = [[big.tile([128, 512], BF16, tag=f"qk{co}_{tq}", name=f"qk{co}_{tq}")
                 for tq in range(NTQ)] for co in range(4)]
        v_t = [big.tile([128, 4 * VW], BF16, tag=f"v{t16}", name=f"v{t16}")
               for t16 in range(16)]
        attn_t = [[big.tile([128, 512], BF16, tag=f"at{j}_{tq}", name=f"at{j}_{tq}")
                   for tq in range(NTQ)] for j in range(2)]
        bqk_sb = big.tile([128, 4], F32, tag="bqk")
        bvbc_sb = big.tile([128, 256], BF16, tag="bvbc")
        bpbc_sb = big.tile([128, C], BF16, tag="bpbc")
        tri2_sb = big.tile([128, 256], BF16, tag="tri2")

        # ---- input DMAs: few big multi-dim descriptors ----
        nc.sync.dma_start(bqk_sb[:], bqk[:])
        nc.sync.dma_start(tri2_sb[:], tri2[:])
        nc.sync.dma_start(bvbc_sb[:], bvbc[:])
        wa3d = wa.rearrange("(k p) c -> p k c", p=128)
        wa3s = wa_sb[:].rearrange("p (k c) -> p k c", c=WAC)
        xt3d = xt.rearrange("(k p) t -> p k t", p=128)
        xt3s = xt_sb[:].rearrange("p (k t) -> p k t", t=T)
        QT = T // 4
        for g in range(4):  # wa + xt q0, interleaved in 2-k-chunk granules
            ks = slice(2 * g, 2 * g + 2)
            nc.sync.dma_start(wa3s[:, ks], wa3d[:, ks])
            nc.sync.dma_start(xt3s[:, ks, 0:QT], xt3d[:, ks, 0:QT])
        nc.sync.dma_start(xt3s[:, :, QT:2 * QT], xt3d[:, :, QT:2 * QT])
        wp3d = wp.rearrange("(k p) c -> p k c", p=128)
        wp3s = wp_sb[:].rearrange("p (k c) -> p k c", c=C)
        nc.sync.dma_start(wp3s[:], wp3d[:])
        nc.sync.dma_start(bpbc_sb[:], bpbc[:])
        nc.sync.dma_start(xt3s[:, :, 2 * QT:3 * QT], xt3d[:, :, 2 * QT:3 * QT])
        nc.sync.dma_start(xt3s[:, :, 3 * QT:T], xt3d[:, :, 3 * QT:T])

        # ones columns of v_ext (d=64 of each head slot)
        for t16 in range(16):
            ones_view = v_t[t16][:].rearrange("p (h d) -> p h d", d=VW)[:, :, 64:VW]
            nc.gpsimd.memset(ones_view, 1.0)

        # ---- chain builders: lists of (cost_ns, emit_fn) ----
        def qk_chain(co, tq):
            st = {}

            def step(k):
                def f():
                    if k == 0:
                        st["p"] = psum.tile([128, 512], F32, tag="mm", bufs=2,
                                            name=f"qk{co}_{tq}")
                    nc.tensor.matmul(
                        st["p"][:],
                        wa_sb[:, k * WAC + co * 128: k * WAC + (co + 1) * 128],
                        xt_sb[:, k * T + tq * 512: k * T + (tq + 1) * 512],
                        start=(k == 0), stop=(k == KT - 1))
                    if k == KT - 1:
                        nc.vector.tensor_scalar_add(
                            qk_t[co][tq][:], st["p"][:], bqk_sb[:, co:co + 1])
                return (COST_QK, f)
            return [step(k) for k in range(KT)]

        def v_chain(t16):
            st = {}

            def step(k):
                def f():
                    if k == 0:
                        st["p"] = psum.tile([128, 256], F32, tag="mm", bufs=2,
                                            name=f"v{t16}")
                    nc.tensor.matmul(
                        st["p"][:],
                        xt_sb[:, k * T + t16 * 128: k * T + (t16 + 1) * 128],
                        wa_sb[:, k * WAC + 512: (k + 1) * WAC],
                        start=(k == 0), stop=(k == KT - 1))
                    if k == KT - 1:
                        out3 = v_t[t16][:].rearrange("p (h d) -> p h d", d=VW)[:, :, 0:64]
                        in3 = st["p"][:].rearrange("p (h d) -> p h d", d=64)
                        b3 = bvbc_sb[:].rearrange("p (h d) -> p h d", d=64)
                        nc.vector.tensor_add(out3, in3, b3)
                return (COST_V, f)
            return [step(k) for k in range(KT)]

        def proj_chain(t16, n):
            st = {}

            def step(kc):
                def f():
                    if kc == 0:
                        st["p"] = psum.tile([128, 512], F32, tag="mm", bufs=2,
                                            name=f"pr{t16}_{n}")
                    nc.tensor.matmul(
                        st["p"][:],
                        attn_t[kc][t16 // 4][:, (t16 % 4) * 128: (t16 % 4 + 1) * 128],
                        wp_sb[:, kc * C + n * 512: kc * C + (n + 1) * 512],
                        start=(kc == 0), stop=(kc == 1))
                    if kc == 1:
                        ysb = work.tile([128, 512], BF16, tag="y")
                        nc.vector.tensor_add(ysb[:], st["p"][:],
                                             bpbc_sb[:, n * 512:(n + 1) * 512])
                        nc.sync.dma_start(
                            y[t16 * 128:(t16 + 1) * 128, n * 512:(n + 1) * 512], ysb[:])
                return (COST_PROJ, f)
            return [step(kc) for kc in range(2)]

        # ---- filler queue with credit-based pulling ----
        fill_q = deque()
        carry = [0.0]

        def pull(budget):
            carry[0] = min(carry[0] + budget, 1400.0)
            while fill_q and fill_q[0][0] <= carry[0]:
                cost, fn = fill_q.popleft()
                fn()
                carry[0] -= cost

        def drain():
            while fill_q:
                fill_q.popleft()[1]()
            carry[0] = 0.0

        def run_now(steps):
            for _, fn in steps:
                fn()

        # ---- attention for one head pair (heads 2j, 2j+1) over one tq block ----
        def pair_attn(j, tqb):
            nkt = 4 * (tqb + 1)
            av_a = psum.tile([VW, 512], F32, tag="av", bufs=2, name=f"av{j}{tqb}a")
            av_b = psum.tile([VW, 512], F32, tag="av", bufs=2, name=f"av{j}{tqb}b")
            s_t, e_t = {}, {}

            def emit_S(kt):
                s = psum.tile([128, 1024], F32, tag="s", bufs=2)
                m = kt - 4 * tqb
                c0s = m * 128 if (m > 0 and tqb > 0) else 0
                for half in range(2):
                    nc.tensor.matmul(
                        s[:, half * 512 + c0s: (half + 1) * 512],
                        qk_t[2 + j][kt // 4][half * 64:(half + 1) * 64,
                                             (kt % 4) * 128: (kt % 4 + 1) * 128],
                        qk_t[j][tqb][half * 64:(half + 1) * 64, c0s:512],
                        start=True, stop=True, tile_position=(64 * half, 0))
                s_t[kt] = s

            def emit_exp(kt):
                e = work.tile([128, 1024], BF16, tag="e", bufs=3)
                nc.scalar.activation(e[:], s_t.pop(kt)[:], AF.Exp, scale=SCALE)
                m = kt - 4 * tqb
                if m >= 0:
                    c0 = m * 128
                    e3 = e[:].rearrange("p (h q) -> p h q", q=512)[:, :, c0:c0 + 128]
                    t3 = tri2_sb[:].rearrange("p (h q) -> p h q", q=128)
                    nc.vector.tensor_mul(e3, e3, t3)
                e_t[kt] = e

            def emit_AV(kt):
                m = kt - 4 * tqb
                c0 = m * 128 if m > 0 else 0
                e = e_t.pop(kt)
                for half, av in ((0, av_a), (1, av_b)):
                    h = 2 * j + half
                    nc.tensor.matmul(
                        av[:, c0:512],
                        v_t[kt][:, h * VW: (h + 1) * VW],
                        e[:, half * 512 + c0: (half + 1) * 512],
                        start=(kt == 0), stop=(kt == nkt - 1))

            emit_S(0)
            emit_exp(0)
            for kt in range(nkt):
                if kt + 1 < nkt:
                    emit_S(kt + 1)
                emit_AV(kt)
                if kt + 1 < nkt:
                    emit_exp(kt + 1)
                pull(STEP_CREDIT)

            # normalize: attn = av[0:64] * 1/av[64]
            for half, av in ((0, av_a), (1, av_b)):
                den = work.tile([1, 512], F32, tag="den", bufs=2)
                nc.vector.tensor_copy(den[:], av[64:VW, :])
                recipf = work.tile([1, 512], F32, tag="recip", bufs=2)
                nc.vector.reciprocal_approx_fast(recipf[:], den[:])
                bcs = work.tile([64, 512], F32, tag="bcs", bufs=2)
                nc.gpsimd.partition_broadcast(bcs[:], recipf[:])
                nc.vector.tensor_mul(
                    attn_t[j][tqb][half * 64:(half + 1) * 64, :],
                    av[0:64, :], bcs[:])

        # ---- schedule ----
        # upfront QKV for tq block 0 (dense PE work during the DMA-bound start)
        run_now(qk_chain(0, 0))
        run_now(qk_chain(2, 0))
        for t16 in range(4):
            run_now(v_chain(t16))
        run_now(qk_chain(1, 0))
        run_now(qk_chain(3, 0))

        # filler release schedule: QKV(nxt) during tqb, proj deferred late
        # (late attention blocks have the most exp-latency to fill)
        proj_release = {2: [0], 3: [1, 2]}
        for tqb in range(NTQ):
            nxt = tqb + 1
            if nxt < NTQ:
                fill_q.extend(qk_chain(0, nxt))
                fill_q.extend(qk_chain(2, nxt))
                for t16 in range(4 * nxt, 4 * nxt + 4):
                    fill_q.extend(v_chain(t16))
            for pb in proj_release.get(tqb, [])[:1]:
                for t16 in range(4 * pb, 4 * pb + 4):
                    for n in range(2):
                        fill_q.extend(proj_chain(t16, n))
            pair_attn(0, tqb)
            if nxt < NTQ:
                fill_q.extend(qk_chain(1, nxt))
                fill_q.extend(qk_chain(3, nxt))
            for pb in proj_release.get(tqb, [])[1:]:
                for t16 in range(4 * pb, 4 * pb + 4):
                    for n in range(2):
                        fill_q.extend(proj_chain(t16, n))
            pair_attn(1, tqb)
            drain()

        # tail: proj of the last tq block; kc0 matmuls (pair-0 attn, long done)
        # fill the PE while the final normalize chain runs.
        tails = [(t16, n) for t16 in range(12, 16) for n in range(2)]
        for i in range(0, 8, 2):
            chains = [proj_chain(t16, n) for t16, n in tails[i:i + 2]]
            for ch in chains:
                ch[0][1]()
            for ch in chains:
                ch[1][1]()

    nc.compile()
    return nc


def _get_nc():
    global _cached_nc
    if _cached_nc is None:
        _cached_nc = _build()
    return _cached_nc


def make_in_maps(x, w_attn, b_attn, w_proj, b_proj):
    BF = mybir.dt.np(BF16)
    x = np.asarray(x, np.float32)
    w_attn = np.asarray(w_attn, np.float32)
    b_attn = np.asarray(b_attn, np.float32)
    w_proj = np.asarray(w_proj, np.float32)
    b_proj = np.asarray(b_proj, np.float32)
    tri = np.triu(np.ones((128, 128), np.float32))
    tri2 = np.tile(tri, (1, 2)).astype(BF)
    in_maps = []
    for core in range(N_CORES):
        b, hg = core // 4, core % 4
        cs = slice(hg * 256, (hg + 1) * 256)
        wq = w_attn[:, cs]
        wk = w_attn[:, 1024 + hg * 256:1024 + (hg + 1) * 256]
        wv = w_attn[:, 2048 + hg * 256:2048 + (hg + 1) * 256]
        wa = np.ascontiguousarray(
            np.concatenate([wq, wk, wv], axis=1)).astype(BF)
        bqk_vec = np.concatenate(
            [b_attn[cs], b_attn[1024 + hg * 256:1024 + (hg + 1) * 256]])
        in_maps.append({
            "xt": np.ascontiguousarray(x[b].T).astype(BF),
            "wa": wa,
            "bqk": np.ascontiguousarray(bqk_vec.reshape(4, 128).T).astype(np.float32),
            "bvbc": np.broadcast_to(
                b_attn[2048 + hg * 256:2048 + (hg + 1) * 256], (128, 256)).astype(BF),
            "wp": np.ascontiguousarray(w_proj[cs, :]).astype(BF),
            "bpbc": np.broadcast_to(b_proj / 4.0, (128, 1024)).astype(BF),
            "tri2": tri2,
        })
    return in_maps


def kernel(x, w_attn, b_attn, w_proj, b_proj):
    in_maps = make_in_maps(x, w_attn, b_attn, w_proj, b_proj)
    nc = _get_nc()
    res = run_bass_kernel_spmd(nc, in_maps, core_ids=list(range(N_CORES)))
    y = np.zeros((B, T, C), np.float32)
    for core in range(N_CORES):
        y[core // 4] += res.results[core]["y"].astype(np.float32)
    return y


# revision 9
# speedup vs baseline: 1.1642x; 1.0210x over previous
"""Causal self-attention (B=2, T=2048, C=1024, 16 heads) on 8 TRN2 NeuronCores.

Sharding: core = b*4 + hg (b data-parallel over batch, hg tensor-parallel over
head groups of 4 heads). Each core computes QKV for its 4 heads, causal
attention, and a partial output projection (its 256 rows of w_proj); the host
sums the 4 partials per batch element and adds b_proj once.

v4 design (fp32r baseline ~197us, v2 ~187us, v3 ~172.6us):
- bf16 storage + matmul operands (fp32 PSUM accumulation).
- S matmuls row-tiled: K=64 per head, two heads run CONCURRENTLY in PE row
  groups (0,0)/(64,0).
- exp on ScalarE (~86us) is the pacing engine; QKV/proj matmuls are spliced
  between S/AV pairs at single-matmul granularity. Fillers live in two
  queues: QKV chains are deadline-bound (must be emitted before the next
  block's attention reads them -> hard-drained at block boundaries), proj
  chains are soft and flow into the late blocks where exp-latency demand is
  highest. proj of block 2 is held back entirely to cover the final
  normalize window.
- Tile hazard tracking follows emission order coarsely, so order of python
  emission is load-bearing throughout.
- b_proj is added on the host (free), so proj PSUM->SBUF moves are pure
  copies; the tail moves run on the then-idle ScalarE.
- Input DMAs are few big multi-dim descriptors (Sync issues each in ~600ns),
  ordered so the first attention block's dependencies land first (wa column
  layout is [q01|k01|q23|k23|v] to make that prefix contiguous).
"""
import numpy as np
from collections import deque
from contextlib import ExitStack

import concourse.bass as bass
import concourse.tile as tile
from concourse import bacc, mybir
from concourse.bass_utils import run_bass_kernel_spmd

F32 = mybir.dt.float32
BF16 = mybir.dt.bfloat16
AF = mybir.ActivationFunctionType

B, T, C = 2, 2048, 1024
N_CORES = 8
KT = 8              # contraction tiles over C (1024/128)
NTQ = 4             # T blocks of 512 (query blocks)
SCALE = 1.0 / 8.0   # 1/sqrt(HEAD_DIM)
WAC = 768           # fused weight cols per k-chunk
VW = 65             # v_ext cols per head: [v(64) | 1]
CO_OFF = {0: 0, 2: 128, 1: 256, 3: 384}  # wa col offset per qk co tile

COST_QK = 213.0
COST_V = 107.0
COST_PROJ = 213.0
STEP_CREDIT = 450.0

_cached_nc = None


def _build():
    nc = bacc.Bacc("TRN2", target_bir_lowering=False, debug=False,
                   enable_asserts=True, num_devices=N_CORES)
    xt = nc.dram_tensor("xt", [C, T], BF16, kind="ExternalInput").ap()
    wa = nc.dram_tensor("wa", [C, WAC], BF16, kind="ExternalInput").ap()
    bqk = nc.dram_tensor("bqk", [128, 4], F32, kind="ExternalInput").ap()
    bvbc = nc.dram_tensor("bvbc", [128, 256], BF16, kind="ExternalInput").ap()
    wp = nc.dram_tensor("wp", [256, C], BF16, kind="ExternalInput").ap()
    tri2 = nc.dram_tensor("tri2", [128, 256], BF16, kind="ExternalInput").ap()
    y = nc.dram_tensor("y", [T, C], BF16, kind="ExternalOutput").ap()

    with tile.TileContext(nc) as tc, ExitStack() as ctx:
        big = ctx.enter_context(tc.tile_pool(name="big", bufs=1))
        work = ctx.enter_context(tc.tile_pool(name="work", bufs=2))
        psum = ctx.enter_context(tc.tile_pool(name="psum", bufs=1, space="PSUM"))

        # ---- persistent SBUF tensors (split per block for precise hazards) ----
        xt_sb = big.tile([128, KT * T], BF16, tag="xt")
        wa_sb = big.tile([128, KT * WAC], BF16, tag="wa")
        wp_sb = big.tile([128, 2 * C], BF16, tag="wp")
        qk_t = [[big.tile([128, 512], BF16, tag=f"qk{co}_{tq}", name=f"qk{co}_{tq}")
                 for tq in range(NTQ)] for co in range(4)]
        v_t = [big.tile([128, 4 * VW], BF16, tag=f"v{t16}", name=f"v{t16}")
               for t16 in range(16)]
        attn_t = [[big.tile([128, 512], BF16, tag=f"at{j}_{tq}", name=f"at{j}_{tq}")
                   for tq in range(NTQ)] for j in range(2)]
        bqk_sb = big.tile([128, 4], F32, tag="bqk")
        bvbc_sb = big.tile([128, 256], BF16, tag="bvbc")
        tri2_sb = big.tile([128, 256], BF16, tag="tri2")

        # ones columns of v_ext (d=64 of each head slot); no input deps
        for t16 in range(16):
            ones_view = v_t[t16][:].rearrange("p (h d) -> p h d", d=VW)[:, :, 64:VW]
            nc.gpsimd.memset(ones_view, 1.0)

        # ---- input DMAs: big descriptors in compute need-order ----
        nc.sync.dma_start(bqk_sb[:], bqk[:])
        nc.sync.dma_start(tri2_sb[:], tri2[:])
        nc.sync.dma_start(bvbc_sb[:], bvbc[:])
        wa3d = wa.rearrange("(k p) c -> p k c", p=128)
        wa3s = wa_sb[:].rearrange("p (k c) -> p k c", c=WAC)
        xt3d = xt.rearrange("(k p) t -> p k t", p=128)
        xt3s = xt_sb[:].rearrange("p (k t) -> p k t", t=T)
        QT = T // 4
        nc.sync.dma_start(wa3s[:, :, 0:256], wa3d[:, :, 0:256])        # q01|k01
        nc.sync.dma_start(xt3s[:, :, 0:QT], xt3d[:, :, 0:QT])
        nc.sync.dma_start(wa3s[:, :, 512:WAC], wa3d[:, :, 512:WAC])    # v
        nc.sync.dma_start(wa3s[:, :, 256:512], wa3d[:, :, 256:512])    # q23|k23
        nc.sync.dma_start(xt3s[:, :, QT:2 * QT], xt3d[:, :, QT:2 * QT])
        wp3d = wp.rearrange("(k p) c -> p k c", p=128)
        wp3s = wp_sb[:].rearrange("p (k c) -> p k c", c=C)
        nc.sync.dma_start(wp3s[:], wp3d[:])
        nc.sync.dma_start(xt3s[:, :, 2 * QT:3 * QT], xt3d[:, :, 2 * QT:3 * QT])
        nc.sync.dma_start(xt3s[:, :, 3 * QT:T], xt3d[:, :, 3 * QT:T])

        # ---- chain builders: lists of (cost_ns, emit_fn) ----
        def qk_chain(co, tq):
            st = {}

            def step(k):
                def f():
                    if k == 0:
                        st["p"] = psum.tile([128, 512], F32, tag="mm", bufs=2,
                                            name=f"qk{co}_{tq}")
                    nc.tensor.matmul(
                        st["p"][:],
                        wa_sb[:, k * WAC + CO_OFF[co]: k * WAC + CO_OFF[co] + 128],
                        xt_sb[:, k * T + tq * 512: k * T + (tq + 1) * 512],
                        start=(k == 0), stop=(k == KT - 1))
                    if k == KT - 1:
                        nc.vector.tensor_scalar_add(
                            qk_t[co][tq][:], st["p"][:], bqk_sb[:, co:co + 1])
                return (COST_QK, f)
            return [step(k) for k in range(KT)]

        def v_chain(t16):
            st = {}

            def step(k):
                def f():
                    if k == 0:
                        st["p"] = psum.tile([128, 256], F32, tag="mm", bufs=2,
                                            name=f"v{t16}")
                    nc.tensor.matmul(
                        st["p"][:],
                        xt_sb[:, k * T + t16 * 128: k * T + (t16 + 1) * 128],
                        wa_sb[:, k * WAC + 512: (k + 1) * WAC],
                        start=(k == 0), stop=(k == KT - 1))
                    if k == KT - 1:
                        out3 = v_t[t16][:].rearrange("p (h d) -> p h d", d=VW)[:, :, 0:64]
                        in3 = st["p"][:].rearrange("p (h d) -> p h d", d=64)
                        b3 = bvbc_sb[:].rearrange("p (h d) -> p h d", d=64)
                        nc.vector.tensor_add(out3, in3, b3)
                return (COST_V, f)
            return [step(k) for k in range(KT)]

        def proj_chain(t16, n, move="v"):
            st = {}

            def step(kc):
                def f():
                    if kc == 0:
                        st["p"] = psum.tile([128, 512], F32, tag="mm", bufs=2,
                                            name=f"pr{t16}_{n}")
                    nc.tensor.matmul(
                        st["p"][:],
                        attn_t[kc][t16 // 4][:, (t16 % 4) * 128: (t16 % 4 + 1) * 128],
                        wp_sb[:, kc * C + n * 512: kc * C + (n + 1) * 512],
                        start=(kc == 0), stop=(kc == 1))
                    if kc == 1:
                        ysb = work.tile([128, 512], BF16, tag="y")
                        if move == "s":
                            nc.scalar.activation(ysb[:], st["p"][:], AF.Copy)
                        else:
                            nc.vector.tensor_copy(ysb[:], st["p"][:])
                        nc.sync.dma_start(
                            y[t16 * 128:(t16 + 1) * 128, n * 512:(n + 1) * 512], ysb[:])
                return (COST_PROJ, f)
            return [step(kc) for kc in range(2)]

        # ---- two filler queues: deadline-bound QKV, soft proj ----
        qkv_q = deque()
        proj_q = deque()
        carry = [0.0]

        def pull(budget):
            carry[0] = min(carry[0] + budget, 1400.0)
            while True:
                q = qkv_q if qkv_q else proj_q
                if not q or q[0][0] > carry[0]:
                    break
                cost, fn = q.popleft()
                fn()
                carry[0] -= cost

        def drain_qkv():
            while qkv_q:
                qkv_q.popleft()[1]()
            carry[0] = 0.0

        def run_now(steps):
            for _, fn in steps:
                fn()

        # ---- attention for one head pair (heads 2j, 2j+1) over one tq block ----
        def pair_attn(j, tqb):
            nkt = 4 * (tqb + 1)
            av_a = psum.tile([VW, 512], F32, tag="av", bufs=2, name=f"av{j}{tqb}a")
            av_b = psum.tile([VW, 512], F32, tag="av", bufs=2, name=f"av{j}{tqb}b")
            s_t, e_t = {}, {}

            def emit_S(kt):
                s = psum.tile([128, 1024], F32, tag="s", bufs=2)
                m = kt - 4 * tqb
                c0s = m * 128 if (m > 0 and tqb > 0) else 0
                for half in range(2):
                    nc.tensor.matmul(
                        s[:, half * 512 + c0s: (half + 1) * 512],
                        qk_t[2 + j][kt // 4][half * 64:(half + 1) * 64,
                                             (kt % 4) * 128: (kt % 4 + 1) * 128],
                        qk_t[j][tqb][half * 64:(half + 1) * 64, c0s:512],
                        start=True, stop=True, tile_position=(64 * half, 0))
                s_t[kt] = s

            def emit_exp(kt):
                e = work.tile([128, 1024], BF16, tag="e", bufs=3)
                s = s_t.pop(kt)
                m = kt - 4 * tqb
                if m >= 2 and tqb > 0:
                    # masked prefix of each half is never read by AV: skip it
                    c0 = m * 128
                    for half in range(2):
                        sl = slice(half * 512 + c0, (half + 1) * 512)
                        nc.scalar.activation(e[:, sl], s[:, sl], AF.Exp, scale=SCALE)
                else:
                    nc.scalar.activation(e[:], s[:], AF.Exp, scale=SCALE)
                if m >= 0:
                    c0 = m * 128
                    e3 = e[:].rearrange("p (h q) -> p h q", q=512)[:, :, c0:c0 + 128]
                    t3 = tri2_sb[:].rearrange("p (h q) -> p h q", q=128)
                    nc.vector.tensor_mul(e3, e3, t3)
                e_t[kt] = e

            def emit_AV(kt):
                m = kt - 4 * tqb
                c0 = m * 128 if m > 0 else 0
                e = e_t.pop(kt)
                for half, av in ((0, av_a), (1, av_b)):
                    h = 2 * j + half
                    nc.tensor.matmul(
                        av[:, c0:512],
                        v_t[kt][:, h * VW: (h + 1) * VW],
                        e[:, half * 512 + c0: (half + 1) * 512],
                        start=(kt == 0), stop=(kt == nkt - 1))

            emit_S(0)
            emit_exp(0)
            for kt in range(nkt):
                if kt + 1 < nkt:
                    emit_S(kt + 1)
                emit_AV(kt)
                if kt + 1 < nkt:
                    emit_exp(kt + 1)
                pull(STEP_CREDIT)

            # normalize: attn = av[0:64] * 1/av[64]; den copy on ScalarE
            for half, av in ((0, av_a), (1, av_b)):
                den = work.tile([1, 512], F32, tag="den", bufs=2)
                nc.scalar.activation(den[:], av[64:VW, :], AF.Copy)
                recipf = work.tile([1, 512], F32, tag="recip", bufs=2)
                nc.vector.reciprocal_approx_fast(recipf[:], den[:])
                bcs = work.tile([64, 512], F32, tag="bcs", bufs=2)
                nc.gpsimd.partition_broadcast(bcs[:], recipf[:])
                nc.vector.tensor_mul(
                    attn_t[j][tqb][half * 64:(half + 1) * 64, :],
                    av[0:64, :], bcs[:])

        # ---- schedule ----
        # upfront QKV for tq block 0 (dense PE work during the DMA-bound start)
        run_now(qk_chain(0, 0))
        run_now(qk_chain(2, 0))
        for t16 in range(4):
            run_now(v_chain(t16))
        run_now(qk_chain(1, 0))
        run_now(qk_chain(3, 0))

        for tqb in range(NTQ):
            nxt = tqb + 1
            if nxt < NTQ:
                qkv_q.extend(qk_chain(0, nxt))
                qkv_q.extend(qk_chain(2, nxt))
                for t16 in range(4 * nxt, 4 * nxt + 4):
                    qkv_q.extend(v_chain(t16))
            pair_attn(0, tqb)
            if nxt < NTQ:
                qkv_q.extend(qk_chain(1, nxt))
                qkv_q.extend(qk_chain(3, nxt))
            pair_attn(1, tqb)
            drain_qkv()
            # proj for this block becomes soft filler (block 2 held for tail)
            if tqb < 2:
                for t16 in range(4 * tqb, 4 * tqb + 4):
                    for n in range(2):
                        proj_q.extend(proj_chain(t16, n))

        # proj(block 2) covers the final normalize window
        for t16 in range(8, 12):
            for n in range(2):
                run_now(proj_chain(t16, n, move="s"))
        while proj_q:
            proj_q.popleft()[1]()
        # tail: proj of the last tq block, PSUM->SBUF moves on idle ScalarE
        for t16 in range(12, 16):
            for n in range(2):
                run_now(proj_chain(t16, n, move="s"))

    nc.compile()
    return nc


def _get_nc():
    global _cached_nc
    if _cached_nc is None:
        _cached_nc = _build()
    return _cached_nc


def make_in_maps(x, w_attn, b_attn, w_proj, b_proj):
    BF = mybir.dt.np(BF16)
    x = np.asarray(x, np.float32)
    w_attn = np.asarray(w_attn, np.float32)
    b_attn = np.asarray(b_attn, np.float32)
    w_proj = np.asarray(w_proj, np.float32)
    tri = np.triu(np.ones((128, 128), np.float32))
    tri2 = np.tile(tri, (1, 2)).astype(BF)
    in_maps = []
    for core in range(N_CORES):
        b, hg = core // 4, core % 4
        cs = slice(hg * 256, (hg + 1) * 256)
        wq = w_attn[:, cs]
        wk = w_attn[:, 1024 + hg * 256:1024 + (hg + 1) * 256]
        wv = w_attn[:, 2048 + hg * 256:2048 + (hg + 1) * 256]
        # col layout [q01|k01|q23|k23|v] so the startup DMA prefix is contiguous
        wa = np.ascontiguousarray(np.concatenate(
            [wq[:, 0:128], wk[:, 0:128], wq[:, 128:256], wk[:, 128:256], wv],
            axis=1)).astype(BF)
        bqk_vec = np.concatenate(
            [b_attn[cs], b_attn[1024 + hg * 256:1024 + (hg + 1) * 256]])
        in_maps.append({
            "xt": np.ascontiguousarray(x[b].T).astype(BF),
            "wa": wa,
            "bqk": np.ascontiguousarray(bqk_vec.reshape(4, 128).T).astype(np.float32),
            "bvbc": np.broadcast_to(
                b_attn[2048 + hg * 256:2048 + (hg + 1) * 256], (128, 256)).astype(BF),
            "wp": np.ascontiguousarray(w_proj[cs, :]).astype(BF),
            "tri2": tri2,
        })
    return in_maps


def kernel(x, w_attn, b_attn, w_proj, b_proj):
    in_maps = make_in_maps(x, w_attn, b_attn, w_proj, b_proj)
    nc = _get_nc()
    res = run_bass_kernel_spmd(nc, in_maps, core_ids=list(range(N_CORES)))
    y = np.zeros((B, T, C), np.float32)
    for core in range(N_CORES):
        y[core // 4] += res.results[core]["y"].astype(np.float32)
    y += np.asarray(b_proj, np.float32)[None, None, :]
    return y
